# revision 45
# baseline (speedup 1.0000x reference)
# Multi-head attention (BS=2, QLEN=2048, DIM=1024, NHEADS=16) on 8 NeuronCores.
#
# Sharding: batch x head-group. Core c handles batch b = c // 4 and head group
# g = c % 4 (4 heads = 256 head-dim columns). Each core computes its 4 heads'
# attention plus the partial output projection (row-parallel Wo); the host sums
# the 4 partials per batch and adds bo.
#
# Mask compaction: the reference replaces masked scores with 1e-10, so every
# masked key position contributes weight exp(1e-10 - C) ~= exp(0 - C) times its
# v row -- identical for all of them up to 1e-10. The whole masked-set effect
# is therefore a per-head rank-1 term the host folds into a per-partition
# correction column (cmc); the device runs QK^T / exp / PV only over the
# compacted unmasked keys (padded to NKPAD, exp-bias -40 on pad rows).
#
# Device algorithm (per core), matmuls bf16 with f32 PSUM accumulation:
#   kT = (Wk_g @ xkm.T)               [256 j, NKPAD tk]  (+bias on DVE drain)
#   qT = (Wq_g/8 @ x_b.T)             [256 j, 2048 tq]
#   v  = (xkm @ Wv_g.T | ones)        [NKPAD tk, 4*65 j']
#   per (tq-block, head pair): sT = kT_h.T @ qT_h; PT = exp(sT + padbias)
#     ctx'_h = v'_h.T @ PT            [65, 1024] at partitions 0..64 (Z at 64)
#   norm: zr = 1/Z via exp(-ln(Zraw + cmcZ)) on ACT (PSUM-direct, column
#         paced, ~2.3us/head) for the blocks whose ctxn gates downstream
#         riders, or DVE's exact-but-slow iterative divide where a whole
#         block of slack hides it; zb = broadcast(zr) via a DRAM bounce
#         (partition-stride-0 DRAM AP) mid-stream, or a PE rank-1 f32
#         ones-matmul for the last block; ctxn = (ctx'+cmc) * zb; head B
#         DMA-shifts to partitions 64..127
#   out += ctxn.T @ Wo_g.T            [tq, 1024] bf16 partial, host-summed
#
# Scheduling notes (what the ~20% over the exp-stream floor costs went to):
# - The exp stream on ACT (72 x [128,1024] tiles ~ 80us) is the global
#   pacer. Scores PSUM slots are PER-HEAD (tag sp0/sp1, bufs=1): head A's
#   scores(mt+1) then only wait exp_A(mt), so the PE runs a full mt ahead
#   and exp_B(mt) -> exp_A(mt+1) is back-to-back (~1.08us cadence).
# - Per-mt emission interleaves heads (scores_A, exp_A, PV_A(mt-1),
#   scores_B, exp_B, PV_B(mt-1)) so no head-B wait ever head-of-line
#   blocks head A's PE work.
# - Leftover projections (v tiles, k-j1/q-j1, q-hi) and the first 6
#   out-proj tiles ride the per-mt PE slack; their full-128-deep matmuls
#   also keep the HAM activity monitor from re-throttling the PE clock to
#   1.2GHz (scores/PV only light up half the array, which HAM's busy
#   criterion does not credit).
# - Engine FIFOs are strict: anything slow on DVE (iterative divide) or
#   ACT (DGE issues whose descriptor ring is full) head-of-line blocks
#   every later op on that engine; the norm chains and DMA issue
#   placement are arranged around that.
#
# Skipping softmax max-subtraction is safe: |scores| < ~5 for this problem.
# A short warmup matmul chain holds the PE p-state ramp while inputs
# stream in. Host pre-packs every operand in SBUF layout so each DMA is a
# plain [128, N] row copy at full bandwidth.

import numpy as np
import ml_dtypes

BS, T, D, H = 2, 2048, 1024, 16
DPH = D // H              # 64
NCORES = 8
GROUPS = 4                # head groups (cores per batch)
HPG = H // GROUPS         # 4 heads per core
JP = HPG * DPH            # 256 head-dim columns per core
JV = HPG * (DPH + 1)      # 260 v columns (with ones column per head)
NJT = JP // 128           # 2 j-tiles
NDT = D // 128            # 8 d-tiles
NTT = T // 128            # 16 t-tiles
NKPAD = 1152              # compacted-key pad (counts are 1002/1034 for seed 0)
PADBIAS = -40.0           # exp(s - 40) == 0 for pad rows

_CACHE = {}


def _build_bass(nkpad):
    import concourse.bass as bass
    import concourse.mybir as mybir
    from concourse.bass import ts, ds
    from concourse.tile import TileContext
    from contextlib import ExitStack

    dt = mybir.dt
    f32, bf16 = dt.float32, dt.bfloat16
    AF = mybir.ActivationFunctionType
    nkt = nkpad // 128
    kchunks = [512] * (nkpad // 512)
    if nkpad % 512:
        kchunks.append(nkpad % 512)

    nc = bass.Bass("TRN2", target_bir_lowering=False, debug=False,
                   num_devices=NCORES)

    # All operands arrive host-packed in SBUF layout: DMA = [128, N] row copy.
    x_t = nc.dram_tensor("x_t", [NDT, 128, T], bf16, kind="ExternalInput").ap()
    wq = nc.dram_tensor("wq", [NJT, 128, NDT * 128], bf16,
                        kind="ExternalInput").ap()
    wk = nc.dram_tensor("wk", [NJT, 128, NDT * 128], bf16,
                        kind="ExternalInput").ap()
    wv = nc.dram_tensor("wv", [128, NDT, JV], bf16, kind="ExternalInput").ap()
    wo = nc.dram_tensor("wo", [128, NJT, D], bf16, kind="ExternalInput").ap()
    bqk = nc.dram_tensor("bqk", [128, 2 * NJT], f32, kind="ExternalInput").ap()
    bvb = nc.dram_tensor("bvb", [128, JV], f32, kind="ExternalInput").ap()
    cmc = nc.dram_tensor("cmc", [128, HPG], f32, kind="ExternalInput").ap()
    padb = nc.dram_tensor("padb", [128, nkt], f32, kind="ExternalInput").ap()
    out = nc.dram_tensor("out", [T, D], bf16, kind="ExternalOutput").ap()
    # DRAM bounce for the softmax denominators: SBUF APs cannot broadcast
    # across partitions, DRAM APs can (partition step 0)
    zs = nc.dram_tensor("zs", [2, NJT, 2, 1024], f32, kind="Internal").ap()

    with TileContext(nc) as tc, ExitStack() as ctx:
        const = ctx.enter_context(tc.sbuf_pool(name="const", bufs=1))

        # ---- warmup: hold the PE p-state ramp while inputs stream in ----
        wu_sb = const.tile([128, 512], bf16)
        nc.vector.memset(wu_sb[:, :], 0.0)
        one_sb = const.tile([65, 64], f32)
        nc.vector.memset(one_sb[:, :], 1.0)
        # dummy activations at t~0: walrus attaches the ~2.7us
        # ACT_TABLE_LOAD (exp+ln set) here, off the exp stream's critical
        # path. ACT is otherwise idle until the first real exp (~20us).
        scr_sb = const.tile([1, 8], f32)
        nc.vector.memset(scr_sb[:, :], 1.0)
        nc.scalar.activation(out=scr_sb[0:1, 0:4], in_=scr_sb[0:1, 0:4],
                             func=AF.Exp)
        nc.scalar.activation(out=scr_sb[0:1, 4:8], in_=scr_sb[0:1, 4:8],
                             func=AF.Ln)

        # ---- loads ----
        # small consts on HWDGE (SP); big streams on SWDGE (gpsimd/Pool).
        bqk_sb = const.tile([128, 2 * NJT], f32)
        nc.sync.dma_start(out=bqk_sb, in_=bqk)
        bvb_sb = const.tile([128, JV], f32)
        nc.sync.dma_start(out=bvb_sb, in_=bvb)
        cmc_sb = const.tile([128, HPG], f32)
        nc.sync.dma_start(out=cmc_sb, in_=cmc)
        padb_sb = const.tile([128, nkt], f32)
        nc.sync.dma_start(out=padb_sb, in_=padb)

        # Ring assignment by first-consumer order. The scalar (ACT) ring
        # takes only the first three transfers: DGE issues block the host
        # engine's instruction FIFO once the descriptor ring fills, and
        # ACT must be free for the exp stream from ~20us on.
        wk_sb = const.tile([128, NJT, NDT, 128], bf16)
        nc.scalar.dma_start(out=wk_sb[:, 0, :, :], in_=wk[0])
        # x arrives host-permuted kept-keys-first: columns 0:nkpad double as
        # the compacted-key view (padb nullifies rows count..nkpad), so the
        # separate xkm stream disappears from the DMA budget entirely.
        # x-lo moves as ONE strided transfer per ring (dtiles d, d+3, d+6
        # via a 3D access pattern) — one 0.6us DGE issue instead of three.
        x_sb = const.tile([128, NDT, T], bf16)
        for dtile in range(NDT):
            eng = [nc.sync, nc.gpsimd, nc.scalar][dtile % 3]
            eng.dma_start(out=x_sb[:, dtile, 0:nkpad],
                          in_=x_t[dtile][:, 0:nkpad])
        wq_sb = const.tile([128, NJT, NDT, 128], bf16)
        nc.sync.dma_start(out=wq_sb[:, 0, :, :], in_=wq[0])
        wv_sb = const.tile([128, NDT, JV], bf16)
        nc.gpsimd.dma_start(out=wv_sb, in_=wv)
        nc.gpsimd.dma_start(out=wk_sb[:, 1, :, :], in_=wk[1])
        nc.sync.dma_start(out=wq_sb[:, 1, :, :], in_=wq[1])
        for dtile in range(NDT):
            eng = nc.sync if dtile % 2 else nc.gpsimd
            eng.dma_start(out=x_sb[:, dtile, nkpad:T],
                          in_=x_t[dtile][:, nkpad:T])
        wo_sb = const.tile([128, NJT, D], bf16)
        nc.gpsimd.dma_start(out=wo_sb, in_=wo)

        qT_sb = const.tile([128, NJT, T], bf16)
        kT_sb = const.tile([128, NJT, nkpad], bf16)
        v_sb = const.tile([128, nkt, JV], bf16)
        ctxn_sb = const.tile([128, NJT, T], bf16)

        # ---- single PSUM pool; everything flows through shared tag rings so
        # there are no pool-closure barriers anywhere. Banks: sp0 2 + sp1 2 +
        # c0 2 + c1 2 = 8. The two score slots are PER-HEAD (bufs=1 each)
        # rather than one shared 2-ring: head A's scores(mt+1) then only wait
        # on head A's exp(mt), so the PE runs a full mt ahead of ACT and the
        # exp stream never sees a scores-latency bubble.
        ps_pool = ctx.enter_context(tc.psum_pool(name="ps", bufs=1))
        wk_pool = ctx.enter_context(tc.sbuf_pool(name="wkp", bufs=1))

        _s_rot = [0]

        def s_tile(name):
            # non-score psum users alternate over the two per-head slots
            _s_rot[0] ^= 1
            return ps_pool.tile([128, 1024], f32, tag=f"sp{_s_rot[0]}",
                                name=name, bufs=1)

        # warmup chain: no data deps, keeps PE busy from t~0 so real matmuls
        # start at full clock (p-state ramps over ~3us of continuous work).
        # keepalive() re-writes the same tile (no ring traffic) to bridge
        # DMA-paced gaps in the prefix that would reset the p-state.
        wps = s_tile("wu")

        def keepalive(n):
            for _ in range(n):
                nc.tensor.matmul(wps[:, 0:512], lhsT=wu_sb[:, 0:128],
                                 rhs=wu_sb[:, :], start=True, stop=True)

        keepalive(16)

        # ---- projections: k/q in 1024-col passes through the "s" ring;
        # all bias drains on DVE so ACT stays free for the exp stream ----
        def kproj_pass(jt, base, width):
            ps = s_tile("kps")
            for dtile in range(NDT):
                for off in range(0, width, 512):
                    chunk = min(512, width - off)
                    nc.tensor.matmul(
                        ps[:, ds(off, chunk)],
                        lhsT=wk_sb[:, jt, dtile, :],
                        rhs=x_sb[:, dtile, ds(base + off, chunk)],
                        start=(dtile == 0), stop=(dtile == NDT - 1))
            nc.vector.tensor_scalar_add(
                out=kT_sb[:, jt, ds(base, width)], in0=ps[:, 0:width],
                scalar1=bqk_sb[:, NJT + jt:NJT + jt + 1])

        def kproj(jt):
            for base in range(0, nkpad, 1024):
                kproj_pass(jt, base, min(1024, nkpad - base))

        def qproj_pass(jt, base, width, drain_eng=None):
            ps = s_tile("qps")
            for dtile in range(NDT):
                for off in range(0, width, 512):
                    chunk = min(512, width - off)
                    nc.tensor.matmul(
                        ps[:, ds(off, chunk)],
                        lhsT=wq_sb[:, jt, dtile, :],
                        rhs=x_sb[:, dtile, ds(base + off, chunk)],
                        start=(dtile == 0), stop=(dtile == NDT - 1))
            (drain_eng or nc.vector).tensor_scalar_add(
                out=qT_sb[:, jt, ds(base, width)], in0=ps[:, 0:width],
                scalar1=bqk_sb[:, jt:jt + 1])

        def vproj_tile(tt):
            ps = s_tile("vps")
            for dtile in range(NDT):
                nc.tensor.matmul(
                    ps[:, 0:JV],
                    lhsT=x_sb[:, dtile, ts(tt, 128)],
                    rhs=wv_sb[:, dtile, :],
                    start=(dtile == 0), stop=(dtile == NDT - 1))
            nc.vector.tensor_add(out=v_sb[:, tt, :], in0=ps[:, 0:JV],
                                 in1=bvb_sb[:, :])

        # prefix: what block (0,0) + the exp stream need up-front. q-j1 /
        # v2.. ride the attention stream.
        kproj(0)
        keepalive(3)
        qproj_pass(0, 0, 1024)
        vproj_tile(0)
        vproj_tile(1)
        kproj(1)

        # ---- attention: one continuous software-pipelined stream across
        # all four (tq-block, head-pair) blocks. Per mt the emission order
        # is scores_A(mt), exp_A(mt), PV_A(mt-1), scores_B(mt), exp_B(mt),
        # PV_B(mt-1): with per-head score slots, head A's work never sits
        # in the PE FIFO behind a head-B wait, so the PE stays a full mt
        # ahead and ACT paces the stream wall-to-wall. Leftover projection
        # and out-projection work rides the per-mt PE slack (~0.5us). ----
        blocks = [(nb, p) for nb in range(2) for p in range(NJT)]
        cps_map = {}

        def emit_scores_head(nb, p, mt, i):
            # head A occupies PE rows 0:63, head B rows 64:127 (the
            # contraction is the 64-wide head dim): emitted back-to-back,
            # the two heads' matmuls run CONCURRENTLY in disjoint
            # row-groups whenever both gates are open (PE-bound
            # stretches), halving the scores cost there.
            lo = 64 * i
            sp = ps_pool.tile([128, 1024], f32, tag=f"sp{i}",
                              name=f"sp{i}", bufs=1)
            for n5 in range(2):
                nc.tensor.matmul(
                    sp[:, ts(n5, 512)],
                    lhsT=kT_sb[lo:lo + 64, p, ts(mt, 128)],
                    rhs=qT_sb[lo:lo + 64, p,
                              ds(nb * 1024 + n5 * 512, 512)],
                    start=True, stop=True)
            return sp

        def emit_exp_head(nb, p, mt, i, sp, pend):
            pt = wk_pool.tile([128, 1024], bf16, tag=f"pt{i}",
                              name=f"pt{i}", bufs=2)
            nc.scalar.activation(
                out=pt[:, :], in_=sp[:, :], func=AF.Exp,
                bias=padb_sb[:, mt:mt + 1])
            if pend is not None:
                emit_pv_head(*pend, i)
            return pt

        def emit_pv_head(nb, p, mt, pts, i):
            # lazy cps alloc: by the first PV of a block, the previous
            # block's normalization t1-add (the banks' last reader) is
            # emitted, so the ring dependency is complete
            if mt == 0:
                cps_map.setdefault((nb, p), {})[i] = ps_pool.tile(
                    [128, 1024], f32, tag=f"c{i}", name=f"c{i}")
            cp = cps_map[(nb, p)][i]
            h = NJT * p + i
            for n5 in range(2):
                nc.tensor.matmul(
                    cp[0:65, ts(n5, 512)],
                    lhsT=v_sb[:, mt, ds(h * 65, 65)],
                    rhs=pts[i][:, ts(n5, 512)],
                    start=(mt == 0), stop=(mt == nkt - 1))
            if mt == nkt - 1 and i == 1 and (nb, p) != blocks[-1]:
                # norms of (0,0)/(0,1) run 1/Z on ACT (~2.3us/head,
                # PSUM-direct): their ctxn gates the q-hi riders'
                # s-slots and the (1,0) out-proj riders, and the ACT
                # chain keeps it off the 13us DVE iterative-divide path.
                # (1,0)'s norm is only consumed by the tail, so it hides
                # on DVE during (1,1) for free.
                cps = [cps_map[(nb, p)][0], cps_map[(nb, p)][1]]
                att_norm(nb, p, cps, recip_on_act=(nb == 0))

        def att_norm(nb, p, cps, recip_on_act=False):
            dmas = {0: nc.sync, 1: nc.sync}
            # free both PSUM accumulators first (next pair's PV waits on
            # them), then finish the normalization chains.
            # DVE lanes cannot shift partitions, so head B routes through
            # SBUF and DMAs to partitions 64..127.
            order = (0, 1)
            t1s = {}
            for i in order:
                h = NJT * p + i
                t1 = wk_pool.tile([65, 1024], f32, tag="t1", bufs=2,
                                  name="t1")
                nc.vector.tensor_scalar_add(
                    out=t1[:, :], in0=cps[i][0:65, :],
                    scalar1=cmc_sb[0:65, h:h + 1])
                t1s[i] = t1
            for i in order:
                t1 = t1s[i]
                dma = dmas[i]
                h = NJT * p + i
                zr = wk_pool.tile([65, 1024], f32, tag="zr", bufs=2,
                                  name="zr")
                if recip_on_act:
                    # 1/Z = exp(-ln(Zraw + cmcZ)) on ACT: 2 column-paced
                    # ops (~2.3us) instead of DVE's 8-iteration divide
                    # (~6.5us/head). The ln reads the PSUM row directly
                    # with the masked-key correction on the bias port, so
                    # it doesn't wait for the DVE t1 chain. Ln and Exp
                    # share one activation table set -> no table reload
                    # against the exp stream.
                    lnz = wk_pool.tile([65, 1024], f32, tag="lnz", bufs=2,
                                       name="lnz")
                    nc.scalar.activation(out=lnz[64:65, :],
                                         in_=cps[i][64:65, :], func=AF.Ln,
                                         bias=cmc_sb[64:65, h:h + 1])
                    nc.scalar.activation(out=zr[64:65, :],
                                         in_=lnz[64:65, :], func=AF.Exp,
                                         scale=-1.0)
                else:
                    nc.vector.reciprocal(out=zr[64:65, :],
                                         in_=t1[64:65, :])
                zb = wk_pool.tile([64, 1024], f32, tag="zb", bufs=2,
                                  name="zb")
                zrow = zs[nb, p, i, :]
                dma.dma_start(out=zrow, in_=zr[64:65, :])
                zbcast = bass.AP(tensor=zrow.tensor, offset=zrow.offset,
                                 ap=[[0, 64], *zrow.ap])
                dma.dma_start(out=zb[:, :], in_=zbcast)
                if i == 0:
                    nc.vector.tensor_mul(
                        out=ctxn_sb[0:64, p, ds(nb * 1024, 1024)],
                        in0=t1[0:64, :], in1=zb[:, :])
                else:
                    tmpb = wk_pool.tile([64, 1024], bf16, tag="tmpb",
                                        bufs=2, name="tmpb")
                    nc.vector.tensor_mul(out=tmpb[:, :], in0=t1[0:64, :],
                                         in1=zb[:, :])
                    dma.dma_start(
                        out=ctxn_sb[64:128, p, ds(nb * 1024, 1024)],
                        in_=tmpb[:, :])

        def out_proj(tt, drain, tag="s"):
            if tag == "s":
                ps = s_tile("ops")
            else:
                ps = ps_pool.tile([128, 1024], f32, tag=tag, name="ops")
            for jt in range(NJT):
                for ot in range(2):
                    nc.tensor.matmul(
                        ps[:, ts(ot, 512)],
                        lhsT=ctxn_sb[:, jt, ts(tt, 128)],
                        rhs=wo_sb[:, jt, ts(ot, 512)],
                        start=(jt == 0), stop=(jt == NJT - 1))
            stage = wk_pool.tile([128, D], bf16, tag="stage", bufs=12,
                                 name="stage")
            if drain == "act":
                # DVE is still normalizing the last head pair: ACT (done
                # with the exp stream) takes the whole drain
                nc.scalar.copy(out=stage[:, :], in_=ps[:, :])
            elif drain == "dve":
                # mid-attention: ACT is the exp pacer, keep it clear
                nc.vector.tensor_copy(out=stage[:, :], in_=ps[:, :])
            else:
                # half-drains on DVE and ACT in parallel keep up with the PE
                nc.vector.tensor_copy(out=stage[:, ts(0, 512)],
                                      in_=ps[:, ts(0, 512)])
                nc.scalar.copy(out=stage[:, ts(1, 512)], in_=ps[:, ts(1, 512)])
            # spread out-DMAs over the DGE queues so transfers pipeline;
            # mid-stream riders stay off sync (norm bounce DMAs live
            # there), the tail rotates all three rings (ACT's DGE is free
            # once the exp stream ends)
            if drain in ("act", "dve"):
                odma = nc.gpsimd
            else:
                odma = [nc.sync, nc.gpsimd, nc.scalar][tt % 3]
            odma.dma_start(out=out[ts(tt, 128), :], in_=stage[:, :])

        pend = None
        for nb, p in blocks:
            for mt in range(nkt):
                sps = {i: emit_scores_head(nb, p, mt, i) for i in (0, 1)}
                pts = {}
                pts[0] = emit_exp_head(nb, p, mt, 0, sps[0], pend)
                pts[1] = emit_exp_head(nb, p, mt, 1, sps[1], pend)
                # riders: leftover projection / out-projection work fills
                # the per-mt PE slack. Full-array 128-deep matmuls also
                # keep the HAM activity monitor from re-throttling the PE
                # clock (scores/PV only light up half the array).
                if (nb, p) == (0, 0):
                    if mt + 2 < nkt:
                        # v tiles 2..8: each ready two pipeline stages
                        # before its PV consumes it
                        vproj_tile(mt + 2)
                    if mt == 2:
                        qproj_pass(1, 0, 512)
                    elif mt == 3:
                        qproj_pass(1, 512, 512)
                elif (nb, p) == (0, 1):
                    if mt < 2:
                        # q-j0 hi: consumed by (1,0)
                        qproj_pass(0, 1024 + 512 * mt, 512)
                    elif mt < 4:
                        # q-j1 hi: consumed by (1,1)
                        qproj_pass(1, 1024 + 512 * (mt - 2), 512)
                elif (nb, p) == (1, 0) and 3 <= mt <= 7:
                    # OP tt0-4: ctxn nb=0 is fully normalized by now
                    # ((0,0)/(0,1) norms went through ACT). mt stops at 7
                    # so the DVE drains clear the FIFO before (1,1) mt0's
                    # norm t1-adds need it (those free the cps banks).
                    out_proj(mt - 3, "dve")
                elif (nb, p) == (1, 1) and mt == 0:
                    # (1,0)'s 13us DVE reciprocal chain occupies DVE for
                    # all of (1,1) — an ACT drain costs the pacer 0.85us
                    # once instead of head-of-line blocking an s-slot
                    out_proj(5, "act")

                pend = (nb, p, mt, pts)
        emit_pv_head(*pend, 0)
        emit_pv_head(*pend, 1)
        nb, p = blocks[-1]
        cps = cps_map[blocks[-1]]
        # last block's normalization avoids the DRAM bounce entirely:
        # 1/Z = exp(-ln Z) runs on ACT (free once the exp stream ends,
        # ~2.3us per head vs DVE's 6.5us iterative divide) during OP
        # tt6-7, then the PE broadcasts 1/Z with a rank-1 f32 ones-matmul
        # into the freed cps banks — engine sems instead of two ~0.9us
        # DMA sem hops.
        t1s, zrs = {}, {}
        for i in (1, 0):
            h = NJT * p + i
            # PSUM-direct ln with the masked-key correction on the bias
            # port: the 1/Z chain starts the moment PV stops, without
            # waiting for the DVE t1 adds (those only feed the ctx rows)
            zr = wk_pool.tile([65, 1024], f32, tag="zrl", bufs=2,
                              name="zrl")
            lnzl = wk_pool.tile([65, 1024], f32, tag="lnzl", bufs=2,
                                name="lnzl")
            nc.scalar.activation(out=lnzl[64:65, :], in_=cps[i][64:65, :],
                                 func=AF.Ln, bias=cmc_sb[64:65, h:h + 1])
            nc.scalar.activation(out=zr[64:65, :], in_=lnzl[64:65, :],
                                 func=AF.Exp, scale=-1.0)
            zrs[i] = zr
        for i in (1, 0):
            h = NJT * p + i
            t1 = wk_pool.tile([65, 1024], f32, tag="t1", bufs=2, name="t1")
            nc.vector.tensor_scalar_add(
                out=t1[:, :], in0=cps[i][0:65, :],
                scalar1=cmc_sb[0:65, h:h + 1])
            t1s[i] = t1
        for tt in range(6, NTT // 2):
            out_proj(tt, "act")
        # rank-1 broadcast: zb[j, tq] = ones[j] * (1/Z)[tq]; the PE is
        # warm from tt6-7, reciprocals long done -> no stall, and the
        # head-B partition-shift DMA flies while tt8+ execute
        zbs = {}
        for i in (1, 0):
            zb = ps_pool.tile([128, 1024], f32, tag=f"c{i}", name="zb")
            for n5 in range(2):
                nc.tensor.matmul(zb[0:64, ts(n5, 512)],
                                 lhsT=one_sb[64:65, :],
                                 rhs=zrs[i][64:65, ts(n5, 512)],
                                 start=True, stop=True)
            zbs[i] = zb
        tmpb = wk_pool.tile([64, 1024], bf16, tag="tmpb", bufs=2,
                            name="tmpb")
        nc.vector.tensor_mul(out=tmpb[:, :], in0=t1s[1][0:64, :],
                             in1=zbs[1][0:64, :])
        nc.sync.dma_start(out=ctxn_sb[64:128, p, ds(nb * 1024, 1024)],
                          in_=tmpb[:, :])
        nc.vector.tensor_mul(out=ctxn_sb[0:64, p, ds(nb * 1024, 1024)],
                             in0=t1s[0][0:64, :], in1=zbs[0][0:64, :])
        tags = ["s", "c0", "c1"]
        for k, tt in enumerate(range(NTT // 2, NTT)):
            out_proj(tt, "split", tags[k % 3])

    _legalize_waits(nc, mybir)
    return nc


def _legalize_waits(nc, mybir):
    """The TRN2 ISA carries one sync-wait per instruction, and this walrus
    build refuses to split multi-wait sync_info itself ("Too many sync wait
    commands"). Hoist all but the last wait onto same-engine NoOp carriers
    placed immediately before the instruction."""
    ctr = [0]

    def fix_block(bb):
        insts = list(bb.instructions)
        out = []
        changed = False
        for inst in insts:
            si = inst.sync_info
            waits = list(si.on_wait) if si is not None and si.on_wait else []
            if len(waits) > 1:
                changed = True
                for wx in waits[:-1]:
                    ctr[0] += 1
                    nop = mybir.InstNoOp(name=f"syncnop-{ctr[0]}", ins=[],
                                         outs=[])
                    nop.engine = inst.engine
                    nop.sync_info = mybir.SyncInfo(on_wait=[wx], on_update=[])
                    out.append(nop)
                inst.sync_info = mybir.SyncInfo(on_wait=[waits[-1]],
                                                on_update=list(si.on_update))
            out.append(inst)
        if changed:
            bb.instructions = out

    def walk(blocks):
        for bb in blocks:
            fix_block(bb)
            try:
                sub = bb.blocks
            except AttributeError:
                sub = None
            if sub:
                walk(sub)

    for f in nc.m.functions:
        walk(f.blocks)


def _get_nc(nkpad):
    key = ("nc", nkpad)
    if key not in _CACHE:
        _CACHE[key] = _build_bass(nkpad)
    return _CACHE[key]


def kernel(**inputs):
    return _run(inputs)[0]


def _run(inputs, trace=False):
    from concourse import bass_utils

    x = np.ascontiguousarray(np.asarray(inputs["input"], dtype=np.float32))
    mask = np.asarray(inputs["mask"])
    Wq = np.asarray(inputs["Wq"], dtype=np.float32)
    bq = np.asarray(inputs["bq"], dtype=np.float32)
    Wk = np.asarray(inputs["Wk"], dtype=np.float32)
    bk = np.asarray(inputs["bk"], dtype=np.float32)
    Wv = np.asarray(inputs["Wv"], dtype=np.float32)
    bv = np.asarray(inputs["bv"], dtype=np.float32)
    Wo = np.asarray(inputs["Wo"], dtype=np.float32)
    bo = np.asarray(inputs["bo"], dtype=np.float32)

    scale = 1.0 / np.sqrt(np.float32(DPH))
    bf16 = ml_dtypes.bfloat16

    counts = [int((mask[b] != 0).sum()) for b in range(BS)]
    nkpad = NKPAD
    while max(counts) > nkpad:
        nkpad += 128

    x_t_b, perm_b, xms_b, padb_b, nm_b = [], [], [], [], []
    for b in range(BS):
        keep = np.nonzero(mask[b] != 0)[0]
        drop = np.nonzero(mask[b] == 0)[0]
        nk = len(keep)
        # kept-keys-first permutation: x columns 0:nkpad double as the
        # compacted-key view on device (padb masks rows nk..nkpad)
        perm = np.concatenate([keep, drop])
        perm_b.append(perm)
        xb_bf = x[b][perm].astype(bf16)                 # [T, D] permuted
        xt = xb_bf.T.reshape(NDT, 128, T)               # [d-tile, p, t]
        x_t_b.append(np.ascontiguousarray(xt))
        xms_b.append(x[b][mask[b] == 0].sum(axis=0, dtype=np.float32))
        pb = np.full((nkpad,), PADBIAS, np.float32)
        pb[:nk] = 0.0
        padb_b.append(np.ascontiguousarray(pb.reshape(nkpad // 128, 128).T))
        nm_b.append(np.float32(T - nk))

    in_maps = []
    for c in range(NCORES):
        b, g = divmod(c, GROUPS)
        js = slice(g * JP, (g + 1) * JP)
        # [jt, p(d within tile), dtile, j] packing for q/k stationary weights
        wq_c = (Wq[js] * scale).T.reshape(NDT, 128, NJT, 128)
        wq_c = np.ascontiguousarray(
            wq_c.transpose(2, 1, 0, 3).reshape(NJT, 128, NDT * 128)
        ).astype(bf16)
        wk_c = Wk[js].T.reshape(NDT, 128, NJT, 128)
        wk_c = np.ascontiguousarray(
            wk_c.transpose(2, 1, 0, 3).reshape(NJT, 128, NDT * 128)
        ).astype(bf16)
        wo_c = np.ascontiguousarray(
            Wo[:, js].T.reshape(NJT, 128, D).transpose(1, 0, 2)
        ).astype(bf16)

        # v': per head 65 columns = [v(64), ones]; PV output rows land at
        # partitions 0..64 with the softmax denominator Z at row 64.
        wv_c = np.zeros((D, JV), np.float32)
        bv_c = np.zeros((JV,), np.float32)
        for h in range(HPG):
            gh = g * HPG + h
            wv_c[:, h * 65:h * 65 + DPH] = Wv[gh * DPH:(gh + 1) * DPH].T
            bv_c[h * 65:h * 65 + DPH] = bv[gh * DPH:(gh + 1) * DPH]
            bv_c[h * 65 + DPH] = 1.0
        # rank-1 masked-key term: sum_masked v' = xms @ wv_c + n_m * bv_c
        cmf_c = (xms_b[b] @ wv_c + nm_b[b] * bv_c).astype(np.float32)
        # per-partition correction column per head: ctx 0..63, Z at 64
        cmc_c = np.zeros((128, HPG), np.float32)
        for h in range(HPG):
            cmc_c[0:65, h] = cmf_c[h * 65:(h + 1) * 65]

        bqk_c = np.zeros((128, 2 * NJT), np.float32)
        for jt in range(NJT):
            bqk_c[:, jt] = (bq[js] * scale)[jt * 128:(jt + 1) * 128]
            bqk_c[:, NJT + jt] = bk[js][jt * 128:(jt + 1) * 128]

        wvp = np.ascontiguousarray(
            wv_c.reshape(NDT, 128, JV).transpose(1, 0, 2)).astype(bf16)
        bvb_c = np.ascontiguousarray(
            np.broadcast_to(bv_c, (128, JV))).astype(np.float32)

        in_maps.append({
            "x_t": x_t_b[b],
            "wq": wq_c, "wk": wk_c, "wv": wvp, "wo": wo_c,
            "bqk": np.ascontiguousarray(bqk_c), "bvb": bvb_c,
            "cmc": np.ascontiguousarray(cmc_c), "padb": padb_b[b],
        })

    nc = _get_nc(nkpad)
    res = bass_utils.run_bass_kernel_spmd(nc, in_maps,
                                          core_ids=list(range(NCORES)),
                                          trace=trace)
    final = np.empty((BS, T, D), np.float32)
    for b in range(BS):
        acc = np.asarray(res.results[b * GROUPS]["out"],
                         dtype=np.float32)
        for g in range(1, GROUPS):
            acc += np.asarray(res.results[b * GROUPS + g]["out"],
                              dtype=np.float32)
        # rows come back in kept-first permuted order
        final[b][perm_b[b]] = acc + bo
    return final, res

